# revision 1
# baseline (speedup 1.0000x reference)
"""Trainium2 Bass kernel for CausalSelfAttention (QAT fake-quant + low-rank
adapters + RMSNorm + partial RoPE + GQA causal attention).

Sharding: 8 cores = 2 (batch) x 4 (kv-head groups). Core c handles batch
b = c // 4 and kv group g = c % 4: q heads 4g..4g+3, kv head g. Each core
computes a partial out-projection (its y-column slice x Wproj column slice);
the host sums the 4 partials per batch element.

All heavy math runs on-device in bf16 with fp32 accumulation. The per-row
int8 fake-quant is computed exactly: the host supplies W/scale (fp32 IEEE
divide, matching jax-on-CPU bit-for-bit); the device rounds to the nearest
even integer (magic-constant add/sub in fp32), the integer weights go
through the matmul exactly (|q| <= 127 is exact in bf16), and the row scale
is applied to the matmul output.

Softmax skips the running-max: rms-normed q/k bound |scores| <= sqrt(128),
so exp never overflows fp32 and softmax(x) == exp(x)/sum(exp(x)) exactly.
Scores are built transposed ([j, i]) so P.T never needs materializing; the
softmax denominators come from an all-ones matmul producing the sums row
broadcast across all partitions.
"""

import sys

sys.path.insert(0, '/opt/trn_rl_repo')

from contextlib import ExitStack

import numpy as np

import concourse.bass as bass
import concourse.bacc as bacc
import concourse.tile as tile
from concourse import mybir
from concourse.bass_utils import run_bass_kernel_spmd
from concourse.masks import make_identity

F32 = mybir.dt.float32
BF16 = mybir.dt.bfloat16
AF = mybir.ActivationFunctionType
ALU = mybir.AluOpType

B, S, DIM = 2, 2048, 2048
NH, NKV = 16, 4
HD = 128
RANK = 16
ROPE_DIMS = 64
HALF = ROPE_DIMS // 2  # 32
BASE = 10000.0
EPS = 1.1920929e-7
EPS128 = 128.0 * EPS
MAGIC = 12582912.0  # 1.5 * 2**23: forces round-to-nearest-even at integer ULP
SQRT_HD = float(np.sqrt(128.0))

NT = S // 128           # 16 token tiles of 128
NM = S // 512           # 4 token macros of 512
ND = DIM // 128         # 16 contraction chunks
QF = 4 * HD             # 512 q features per core
KF = HD                 # 128 kv features per core
QKF = QF + KF           # 640 = q+k fused width for norm/rope batching
ZW = 3 * RANK           # 48 = packed q/k/v adapter rank


def _headbc(ap, nheads):
    """View a [128, 32] AP as [128, nheads, 32] with zero head stride."""
    return bass.AP(tensor=ap.tensor, offset=ap.offset,
                   ap=[list(ap.ap[0]), [0, nheads], list(ap.ap[1])])


def build_program():
    nc = bacc.Bacc(None, target_bir_lowering=False)

    xT = nc.declare_dram_parameter("xT", [DIM, S], BF16, isOutput=False)
    wq = nc.declare_dram_parameter("wq", [DIM, QF], F32, isOutput=False)
    wk = nc.declare_dram_parameter("wk", [DIM, KF], F32, isOutput=False)
    wv = nc.declare_dram_parameter("wv", [DIM, KF], F32, isOutput=False)
    wp = nc.declare_dram_parameter("wp", [QF, DIM], F32, isOutput=False)
    sq = nc.declare_dram_parameter("sq", [1, QF], F32, isOutput=False)
    sk = nc.declare_dram_parameter("sk", [1, KF], F32, isOutput=False)
    sv = nc.declare_dram_parameter("sv", [1, KF], F32, isOutput=False)
    spt = nc.declare_dram_parameter("spt", [128, ND], F32, isOutput=False)
    acat = nc.declare_dram_parameter("acat", [DIM, ZW], BF16, isOutput=False)
    qb = nc.declare_dram_parameter("qb", [RANK, QF], BF16, isOutput=False)
    kb = nc.declare_dram_parameter("kb", [RANK, KF], BF16, isOutput=False)
    vb = nc.declare_dram_parameter("vb", [RANK, KF], BF16, isOutput=False)
    pa = nc.declare_dram_parameter("pa", [QF, RANK], BF16, isOutput=False)
    pb = nc.declare_dram_parameter("pb", [RANK, DIM], BF16, isOutput=False)
    cs = nc.declare_dram_parameter("cs", [S, HALF], F32, isOutput=False)
    sn = nc.declare_dram_parameter("sn", [S, HALF], F32, isOutput=False)
    gn = nc.declare_dram_parameter("gn", [1, 4], F32, isOutput=False)
    outT = nc.declare_dram_parameter("outT", [DIM, S], F32, isOutput=True)

    with tile.TileContext(nc) as tc:
        cstack = ExitStack()
        const = cstack.enter_context(tc.tile_pool(name="const", bufs=1))

        ident = const.tile([128, 128], BF16)
        make_identity(nc, ident)
        ones_t = const.tile([128, 128], BF16)
        nc.vector.memset(ones_t, 1.0)
        eps_t = const.tile([128, 1], F32)
        nc.vector.memset(eps_t, EPS128)
        magic_w = const.tile([128, 1024], F32)
        nc.vector.memset(magic_w, MAGIC)
        magic_t = const.tile([128, 1], F32)
        nc.vector.memset(magic_t, MAGIC)
        nmagic_t = const.tile([128, 1], F32)
        nc.vector.memset(nmagic_t, -MAGIC)

        # broadcast fake-quant scale rows across partitions (loaded later)
        sqb = const.tile([128, QF], F32)
        skb = const.tile([128, KF], F32)
        svb = const.tile([128, KF], F32)
        sptile = const.tile([128, ND], F32)
        gainb = const.tile([128, 4], F32)
        gmul = const.tile([128, 5], F32)

        cos_t = const.tile([128, NT, HALF], F32)
        sin_t = const.tile([128, NT, HALF], F32)

        # ---- quantized main weights (rounded on device); k|v|z fused ----
        wq_i = const.tile([128, ND, QF], BF16)
        wkvz_i = const.tile([128, ND, 2 * KF + ZW], BF16)
        wp_i = const.tile([128, QF // 128, DIM], BF16)
        qb_i = const.tile([RANK, QF], BF16)
        kb_i = const.tile([RANK, KF], BF16)
        vb_i = const.tile([RANK, KF], BF16)
        pa_i = const.tile([128, QF // 128, RANK], BF16)
        pb_i = const.tile([RANK, DIM], BF16)

        wstack = ExitStack()
        wstage = wstack.enter_context(tc.tile_pool(name="wstage", bufs=2))

        def round_quant(dst, src_ap, width, dma=None, engine="dve"):
            stg = wstage.tile([128, width], F32, tag="stg",
                              name="stg", padded_shape=[128, 1024])
            (dma or nc.gpsimd).dma_start(out=stg, in_=src_ap)
            if engine == "dve":
                nc.vector.scalar_tensor_tensor(out=dst, in0=stg, scalar=MAGIC,
                                               in1=magic_w[:, 0:width],
                                               op0=ALU.add, op1=ALU.subtract)
            else:
                nc.scalar.activation(out=stg, in_=stg, func=AF.Identity,
                                     bias=magic_t)
                nc.scalar.activation(out=dst, in_=stg, func=AF.Identity,
                                     bias=nmagic_t)

        # ---- resident activation tensors ----
        qT = [const.tile([128, S], BF16, name=f"qT{h}", tag=f"qT{h}")
              for h in range(4)]
        kT = const.tile([128, S], BF16)
        vres = const.tile([128, NT, HD], BF16)
        yT = [const.tile([128, S], BF16, name=f"yT{h}", tag=f"yT{h}")
              for h in range(4)]
        zp_sb = const.tile([RANK, S], BF16)

        # ================= phase B: projections + norm + rope =============
        bstack = ExitStack()
        xstage = bstack.enter_context(tc.tile_pool(name="xstage", bufs=2 * ND + 2))
        bsb = bstack.enter_context(tc.tile_pool(name="bsb", bufs=3))
        ps_q = bstack.enter_context(tc.tile_pool(name="ps_q", bufs=3, space="PSUM"))
        ps_kv = bstack.enter_context(tc.tile_pool(name="ps_kv", bufs=2, space="PSUM"))
        ps_tp = bstack.enter_context(tc.tile_pool(name="ps_tp", bufs=3, space="PSUM"))

        def emit_closure(st):
            pq, pkv, tt = st
            # transpose z -> zq/zk/zv [16, 128]
            z16 = bsb.tile([128, ZW], BF16, tag="z16", name="z16")
            nc.vector.tensor_copy(out=z16, in_=pkv[:, 2 * KF:])
            zq = bsb.tile([RANK, 128], BF16, tag="zq", name="zq")
            zk = bsb.tile([RANK, 128], BF16, tag="zk", name="zk")
            zv = bsb.tile([RANK, 128], BF16, tag="zv", name="zv")
            for zi, zdst in enumerate((zq, zk, zv)):
                ztp = ps_tp.tile([128, 128], BF16, name="ztp",
                                 tag="tp")[0:RANK, :]
                nc.tensor.transpose(
                    ztp, z16[:, zi * RANK:(zi + 1) * RANK], ident)
                nc.scalar.activation(out=zdst, in_=ztp, func=AF.Copy)
            # adapters accumulate into the (stopped) psum regions
            nc.tensor.matmul(pq, zq, qb_i, start=False, stop=True)
            nc.tensor.matmul(pkv[:, 0:KF], zk, kb_i, start=False,
                             stop=False, skip_group_check=True)
            nc.tensor.matmul(pkv[:, KF:2 * KF], zv, vb_i, start=False,
                             stop=False, skip_group_check=True)

            # fused q|k eviction with fq scales -> qk [128, 640] f32
            qk = bsb.tile([128, QKF], F32, tag="qk", name="qk")
            nc.vector.tensor_mul(qk[:, 0:QF], pq, sqb)
            nc.vector.tensor_mul(qk[:, QF:QKF], pkv[:, 0:KF], skb)
            nc.vector.tensor_mul(vres[:, tt, :], pkv[:, KF:2 * KF], svb)

            # batched rms-norm stats: cols 0..3 q heads, col 4 k
            stats = bsb.tile([128, 5], F32, tag="stats", name="stats")
            sqscr = bsb.tile([128, HD], F32, tag="sqscr", name="sqscr")
            for c in range(5):
                nc.scalar.activation(out=sqscr,
                                     in_=qk[:, c * HD:(c + 1) * HD],
                                     func=AF.Square,
                                     accum_out=stats[:, c:c + 1])
            nc.scalar.activation(out=stats, in_=stats, func=AF.Sqrt,
                                 bias=eps_t)
            nc.vector.reciprocal_approx_fast(out=stats, in_=stats)
            nc.vector.tensor_mul(stats, stats, gmul)

            # batched in-place rope over the 5 fused heads
            q5 = qk.rearrange("p (h c) -> p h c", h=5)
            x1 = q5[:, :, 0:HALF]
            x2 = q5[:, :, HALF:ROPE_DIMS]
            cbc = _headbc(cos_t[:, tt, :], 5)
            sbc = _headbc(sin_t[:, tt, :], 5)
            t1 = bsb.tile([128, 5, HALF], F32, tag="t1", name="t1")
            t2 = bsb.tile([128, 5, HALF], F32, tag="t2", name="t2")
            t3 = bsb.tile([128, 5, HALF], F32, tag="t3", name="t3")
            t4 = bsb.tile([128, 5, HALF], F32, tag="t4", name="t4")
            nc.vector.tensor_mul(t1, x1, cbc)
            nc.vector.tensor_mul(t2, x2, sbc)
            nc.vector.tensor_mul(t3, x2, cbc)
            nc.vector.tensor_mul(t4, x1, sbc)
            nc.vector.tensor_add(x1, t1, t2)
            nc.vector.tensor_sub(x2, t3, t4)

            # per-head normalization -> bf16, then transpose to [hd, t]
            qkf = bsb.tile([128, QKF], BF16, tag="qkf", name="qkf")
            for c in range(5):
                nc.vector.tensor_scalar(
                    out=qkf[:, c * HD:(c + 1) * HD],
                    in0=qk[:, c * HD:(c + 1) * HD],
                    scalar1=stats[:, c:c + 1], scalar2=None, op0=ALU.mult)
            for c in range(5):
                tp = ps_tp.tile([128, 128], BF16, name="tp", tag="tp")
                nc.tensor.transpose(tp, qkf[:, c * HD:(c + 1) * HD], ident)
                dst = kT if c == 4 else qT[c]
                nc.scalar.activation(out=dst[:, tt * 128:(tt + 1) * 128],
                                     in_=tp, func=AF.Copy)

        pending = None
        for m in range(NM):
            xts = []
            for d in range(ND):
                xf = xstage.tile([128, 512], BF16, tag="xf", name="xf")
                nc.sync.dma_start(out=xf, in_=xT[d * 128:(d + 1) * 128,
                                               m * 512:(m + 1) * 512])
                xts.append(xf)
                # interleave one-time weight prep with the x stream so the
                # first matmuls start as soon as possible
                if m == 0:
                    nc.sync.dma_start(out=wkvz_i[:, d, 2 * KF:],
                                      in_=acat[d * 128:(d + 1) * 128, :])
                    round_quant(wq_i[:, d, :], wq[d * 128:(d + 1) * 128, :], QF)
                    round_quant(wkvz_i[:, d, 0:KF],
                                wk[d * 128:(d + 1) * 128, :], KF,
                                dma=nc.scalar)
                    round_quant(wkvz_i[:, d, KF:2 * KF],
                                wv[d * 128:(d + 1) * 128, :], KF,
                                dma=nc.scalar)
                elif m == 1 and d < 8:
                    dd, half = divmod(d, 2)
                    hsl = slice(half * 1024, (half + 1) * 1024)
                    round_quant(wp_i[:, dd, hsl],
                                wp[dd * 128:(dd + 1) * 128, hsl], 1024,
                                dma=nc.scalar)
            if m == 0:
                # small constants, needed only after the first d-loop
                nc.sync.dma_start(out=sqb, in_=sq[:, :].to_broadcast([128, QF]))
                nc.sync.dma_start(out=skb, in_=sk[:, :].to_broadcast([128, KF]))
                nc.sync.dma_start(out=svb, in_=sv[:, :].to_broadcast([128, KF]))
                nc.sync.dma_start(out=sptile, in_=spt[:, :])
                nc.sync.dma_start(out=gainb, in_=gn[:, :].to_broadcast([128, 4]))
                nc.vector.tensor_copy(out=gmul[:, 0:4], in_=gainb)
                nc.vector.memset(gmul[:, 4:5], SQRT_HD)
                nc.sync.dma_start(
                    out=cos_t, in_=cs[:, :].rearrange("(a p) d -> p a d", p=128))
                nc.sync.dma_start(
                    out=sin_t, in_=sn[:, :].rearrange("(a p) d -> p a d", p=128))
                nc.sync.dma_start(out=qb_i, in_=qb[:, :])
                nc.sync.dma_start(out=kb_i, in_=kb[:, :])
                nc.sync.dma_start(out=vb_i, in_=vb[:, :])
                nc.sync.dma_start(
                    out=pa_i, in_=pa[:, :].rearrange("(a p) c -> p a c", p=128))
                nc.sync.dma_start(out=pb_i, in_=pb[:, :])
            for tsub in range(4):
                tt = m * 4 + tsub
                tsl = slice(tsub * 128, (tsub + 1) * 128)
                pq = ps_q.tile([128, QF], F32, name="pq")
                pkv = ps_kv.tile([128, 2 * KF + ZW], F32, name="pkv")
                for d in range(ND):
                    lhs = xts[d][:, tsl]
                    nc.tensor.matmul(pq, lhs, wq_i[:, d, :],
                                     start=(d == 0), stop=False)
                    nc.tensor.matmul(pkv, lhs, wkvz_i[:, d, :],
                                     start=(d == 0), stop=(d == ND - 1))
                if pending is not None:
                    emit_closure(pending)
                pending = (pq, pkv, tt)
        emit_closure(pending)
        bstack.close()
        wstack.close()

        # ============ phase C+D: causal attention + out-projection ========
        adstack = ExitStack()
        epool = adstack.enter_context(tc.tile_pool(name="epool", bufs=NT + 2))
        csb = adstack.enter_context(tc.tile_pool(name="csb", bufs=3))
        dsb = adstack.enter_context(tc.tile_pool(name="dsb", bufs=4))
        ps_s = adstack.enter_context(tc.tile_pool(name="ps_s", bufs=4, space="PSUM"))
        ps_y = adstack.enter_context(tc.tile_pool(name="ps_y", bufs=2, space="PSUM"))
        ps_o = adstack.enter_context(tc.tile_pool(name="ps_o", bufs=2, space="PSUM"))

        def emit_outproj(m, ocs):
            isl = slice(m * 512, (m + 1) * 512)
            for oc in ocs:
                osl = slice(oc * 128, (oc + 1) * 128)
                po = ps_o.tile([128, 512], F32, name="po", tag="po")
                for fc in range(4):
                    nc.tensor.matmul(po, wp_i[:, fc, osl], yT[fc][:, isl],
                                     start=(fc == 0), stop=False)
                nc.tensor.matmul(po, pb_i[:, osl], zp_sb[:, isl],
                                 start=False, stop=True)
                osb = dsb.tile([128, 512], F32, tag="osb")
                nc.vector.tensor_scalar(out=osb, in0=po,
                                        scalar1=sptile[:, oc:oc + 1],
                                        scalar2=None, op0=ALU.mult)
                nc.sync.dma_start(out=outT[osl, isl], in_=osb)

        for m in range(NM):
            isl = slice(m * 512, (m + 1) * 512)
            nj = 4 * (m + 1)
            # diagonal blocks first so their exp+mask hides behind the rest
            jorder = list(range(4 * m, 4 * m + 4)) + list(range(0, 4 * m))
            for h in range(4):
                etiles = {}
                for jc in jorder:
                    pscore = ps_s.tile([128, 512], F32, name="pscore", tag="ps")
                    nc.tensor.matmul(pscore, kT[:, jc * 128:(jc + 1) * 128],
                                     qT[h][:, isl], start=True, stop=True)
                    e = epool.tile([128, 512], BF16, name="e", tag="e")
                    nc.scalar.activation(out=e, in_=pscore, func=AF.Exp)
                    if jc >= 4 * m:
                        # causal: zero entries with j > i after exp
                        nc.gpsimd.affine_select(
                            out=e, in_=e, compare_op=ALU.is_ge, fill=0.0,
                            base=m * 512 - jc * 128, channel_multiplier=-1,
                            pattern=[[1, 512]])
                    etiles[jc] = e
                py = ps_y.tile([128, 512], F32, name="py", tag="py")
                pn = ps_o.tile([128, 512], F32, name="pn", tag="po")
                for i, jc in enumerate(jorder):
                    nc.tensor.matmul(py, vres[:, jc, :], etiles[jc],
                                     start=(i == 0), stop=(i == nj - 1))
                for i, jc in enumerate(jorder):
                    # all-ones stationary: every psum row = sum over j
                    nc.tensor.matmul(pn, ones_t, etiles[jc],
                                     start=(i == 0), stop=(i == nj - 1))
                inv = csb.tile([128, 512], F32, tag="inv")
                nc.vector.reciprocal_approx_fast(out=inv, in_=pn)
                nc.vector.tensor_mul(yT[h][:, isl], py, inv)
                # interleave the previous macro's out-projection between heads
                if m >= 1:
                    emit_outproj(m - 1, range(4 * h, 4 * (h + 1)))
            # adapter contraction for this macro (z_proj = y @ proj_A)
            pzp = ps_o.tile([128, 512], F32, name="pzp", tag="po")
            for fc in range(4):
                nc.tensor.matmul(pzp[0:RANK, :], pa_i[:, fc, :], yT[fc][:, isl],
                                 start=(fc == 0), stop=(fc == 3))
            nc.scalar.activation(out=zp_sb[:, isl], in_=pzp[0:RANK, :],
                                 func=AF.Copy)

        emit_outproj(NM - 1, range(ND))
        adstack.close()
        cstack.close()

    nc.finalize()
    return nc


def make_in_maps(x, Wq, Wk, Wv, Wproj, q_gain, q_A, q_B, k_A, k_B, v_A, v_B,
                 proj_A, proj_B):
    """Shard the full inputs into the 8 per-core input maps (host side)."""
    import ml_dtypes
    f32 = np.float32
    bf16 = ml_dtypes.bfloat16
    x, Wq, Wk, Wv, Wproj, q_gain = (np.asarray(a, f32) for a in
                                    (x, Wq, Wk, Wv, Wproj, q_gain))
    q_A, q_B, k_A, k_B, v_A, v_B, proj_A, proj_B = (
        np.asarray(a, f32) for a in (q_A, q_B, k_A, k_B, v_A, v_B,
                                     proj_A, proj_B))

    def fq_scale(w):
        return np.maximum(np.max(np.abs(w), axis=1) / f32(127.0),
                          f32(1.0 / 127.0)).astype(f32)

    scale_q = fq_scale(Wq)
    scale_k = fq_scale(Wk)
    scale_v = fq_scale(Wv)
    scale_p = fq_scale(Wproj)

    # rope tables in fp32, matching reference.rope_tables
    inv_freq = (f32(1.0) / (f32(BASE) ** (np.arange(0, ROPE_DIMS, 2,
                dtype=f32) / f32(ROPE_DIMS)))).astype(f32)
    t = np.arange(S, dtype=f32)
    freqs = np.outer(t, inv_freq).astype(f32)
    cos = np.cos(freqs).astype(f32)
    sin = np.sin(freqs).astype(f32)

    wq_q = (Wq / scale_q[:, None]).astype(f32)
    wk_q = (Wk / scale_k[:, None]).astype(f32)
    wv_q = (Wv / scale_v[:, None]).astype(f32)
    wp_q = (Wproj / scale_p[:, None]).astype(f32)
    acat_b = np.concatenate([q_A, k_A, v_A], axis=1).astype(bf16)

    in_maps = []
    for c in range(8):
        b, g = divmod(c, 4)
        fq0, fq1 = 512 * g, 512 * (g + 1)
        fk0, fk1 = 128 * g, 128 * (g + 1)
        in_maps.append({
            "xT": np.ascontiguousarray(x[b].T).astype(bf16),
            "wq": np.ascontiguousarray(wq_q[fq0:fq1, :].T),
            "wk": np.ascontiguousarray(wk_q[fk0:fk1, :].T),
            "wv": np.ascontiguousarray(wv_q[fk0:fk1, :].T),
            "wp": np.ascontiguousarray(wp_q[:, fq0:fq1].T),
            "sq": np.ascontiguousarray(scale_q[None, fq0:fq1]),
            "sk": np.ascontiguousarray(scale_k[None, fk0:fk1]),
            "sv": np.ascontiguousarray(scale_v[None, fk0:fk1]),
            "spt": np.ascontiguousarray(scale_p.reshape(ND, 128).T),
            "acat": acat_b,
            "qb": (q_B[:, fq0:fq1] / scale_q[None, fq0:fq1]).astype(bf16),
            "kb": (k_B[:, fk0:fk1] / scale_k[None, fk0:fk1]).astype(bf16),
            "vb": (v_B[:, fk0:fk1] / scale_v[None, fk0:fk1]).astype(bf16),
            "pa": np.ascontiguousarray(proj_A[fq0:fq1, :]).astype(bf16),
            "pb": (proj_B / scale_p[None, :]).astype(bf16),
            "cs": cos,
            "sn": sin,
            "gn": np.ascontiguousarray(q_gain[None, 4 * g:4 * (g + 1)]),
        })
    return in_maps


_PROGRAM = None


def kernel(**inputs):
    global _PROGRAM
    if _PROGRAM is None:
        _PROGRAM = build_program()
    in_maps = make_in_maps(**inputs)
    res = run_bass_kernel_spmd(_PROGRAM, in_maps, core_ids=list(range(8)))
    out = np.empty((B, S, DIM), np.float32)
    for b in range(B):
        acc = res.results[4 * b]["outT"].astype(np.float32).copy()
        for g in range(1, 4):
            acc += res.results[4 * b + g]["outT"]
        out[b] = acc.T
    return out

